# revision 13
# baseline (speedup 1.0000x reference)
"""Bootstrapped cross-entropy on 8 Trainium2 NeuronCores.

Strategy (data-parallel over batch B=8, one image per core):
  Launch 1 (per core): per-pixel CE loss for its image.
    - pixels live on 128 "pixel-row" partitions x 4096 free (wide layout);
      compute chunks cover 32 pixel rows x a class group (4+4+4+4+3=19)
      so SBUF chunk tiles are [128 (row x class), F] with F=512.
    - exp on ACT; class-sum via block-diagonal ones matmuls accumulated
      in PSUM quadrants (PE tile_position); pred[target] gather as
      (t_bcast == class_id) * pred fused on DVE (scalar_tensor_tensor);
      target broadcast across class partitions via a small K=32 matmul.
  Host: merge 8 loss shards, exact k-th largest threshold via
    np.partition (selection only; all O(N) arithmetic on device).
  Launch 2 (per core): masked sum + count at the shared threshold
    (the distributed masked mean), combined on host.
"""

import sys

if "/opt/trn_rl_repo" not in sys.path:
    sys.path.insert(0, "/opt/trn_rl_repo")

import numpy as np

import bass_rust
import concourse.bass as bass
import concourse.mybir as mybir
from concourse.tile import TileContext
from concourse.vector_clock import ScopedClock
from concourse.bass_utils import run_bass_kernel_spmd

FP32 = mybir.dt.float32
BF16 = mybir.dt.bfloat16
I32 = mybir.dt.int32
U8 = mybir.dt.uint8
AF = mybir.ActivationFunctionType
OP = mybir.AluOpType
AX = mybir.AxisListType

K_FRAC = 0.15
MOMENTUM = 0.99998
B, C, H, W = 8, 19, 512, 1024
P = 128                      # SBUF partitions (pixel rows)
FT = (H * W) // P            # free elements per partition per core (4096)
RB = 32                      # pixel rows per chunk (one PE quadrant)
NG = 5                       # class groups of 4 (bases 0,4,8,12,15; class 15
CB = [0, 4, 8, 12, 15]       # is read twice, the duplicate zero-weighted)


_WSPLIT_N = [0]


def _cap_sync_waits(nc, max_waits: int = 1):
    """Walrus rejects instructions carrying more than a couple of sem
    waits.  Hoist excess waits onto injected same-engine NoOps placed
    immediately before the instruction (engines dispatch in order, so
    the NoOp's wait gates the original instruction)."""
    for fn in nc.m.functions:
        for bb in fn.blocks:
            out = []
            for inst in bb.instructions:
                si = inst.sync_info
                waits = list(si.on_wait) if si and si.on_wait else []
                if len(waits) > max_waits:
                    upd = list(si.on_update) if si and si.on_update else []
                    extra, keep = waits[:-max_waits], waits[-max_waits:]
                    for i in range(0, len(extra), max_waits):
                        _WSPLIT_N[0] += 1
                        nop = bass_rust.InstNoOp(
                            name=f"I-wsplit-{_WSPLIT_N[0]}", ins=[], outs=[])
                        nop.engine = inst.engine
                        nop.sync_info = bass_rust.SyncInfo(
                            on_wait=extra[i:i + max_waits], on_update=[])
                        out.append(nop)
                    inst.sync_info = bass_rust.SyncInfo(
                        on_wait=keep, on_update=upd)
                out.append(inst)
            bb.instructions = out


def _blockdiag(nc, pool, kp, g, dtype=BF16):
    """[kp, kp//g] tile: 1{k//g == m} (ones block-diagonal), plus f32 copy."""
    m = kp // g
    f = pool.tile([kp, m], FP32, tag=f"bdf_{kp}_{g}")
    nc.vector.memset(f[:, :], 1.0)
    nc.gpsimd.affine_select(f[:, :], f[:, :], pattern=[[-g, m]], base=0,
                            channel_multiplier=1, compare_op=OP.is_ge, fill=0.0)
    nc.gpsimd.affine_select(f[:, :], f[:, :], pattern=[[g, m]], base=(g - 1),
                            channel_multiplier=-1, compare_op=OP.is_ge, fill=0.0)
    b = pool.tile([kp, m], dtype, tag=f"bd_{kp}_{g}")
    nc.vector.tensor_copy(b[:, :], f[:, :])
    return b, f


def _mod_col(nc, pool, kp, g, bd_f):
    """[kp, 1] f32 tile holding k % g (via sum((k-g*m) * blockdiag))."""
    m = kp // g
    io = pool.tile([kp, m], I32, tag=f"iok_{kp}_{g}")
    nc.gpsimd.iota(io[:, :], pattern=[[-g, m]], base=0, channel_multiplier=1)
    iof = pool.tile([kp, m], FP32, tag=f"iof_{kp}_{g}")
    nc.vector.tensor_copy(iof[:, :], io[:, :])
    nc.vector.tensor_mul(iof[:, :], iof[:, :], bd_f[:, :])
    col = pool.tile([kp, 1], FP32, tag=f"mod_{kp}_{g}")
    nc.vector.reduce_sum(col[:, :], iof[:, :], axis=AX.X)
    return col


def build_ce_nc(F: int = 512, S: int = FT // 512, cap_waits: bool = True):
    """CE-loss program for one core: pred [C, P*S*F] f32, tgt [P, S*F] i32
    -> loss [P, S*F] f32.  Pixel (p, f) of the wide layout is element
    p*(S*F)+f of the flat image."""
    free_total = S * F
    npix = P * free_total
    nc = bass.Bass()
    pred_d = nc.dram_tensor("pred", [C, npix], FP32, kind="ExternalInput")
    tgt_d = nc.dram_tensor("tgt", [P, free_total], I32, kind="ExternalInput")
    loss_d = nc.dram_tensor("loss", [P, free_total], FP32, kind="ExternalOutput")

    # per class-group view: (p32, pl, ci, s, f) with classes CB[cg]..CB[cg]+4
    vg = [pred_d[CB[cg]:CB[cg] + 4, :].rearrange(
        "ci (p32 pl s f) -> p32 pl ci s f",
        p32=P // RB, pl=RB, s=S, f=F) for cg in range(NG)]

    with TileContext(nc, pool_alloc_mode="queue") as tc:
        with (
            tc.tile_pool(name="const", bufs=1) as cpool,
            tc.tile_pool(name="tgtp", bufs=1) as tpool,
            tc.tile_pool(name="pred", bufs=5) as predpool,
            tc.tile_pool(name="eprod", bufs=6) as epool,
            tc.tile_pool(name="out", bufs=3) as opool,
            tc.tile_pool(name="psum_acc", bufs=2, space="PSUM") as psacc,
        ):
            # ---- one-time constants ----
            bd4, bd4_f = _blockdiag(nc, cpool, P, 4)      # [128, 32]
            # last group: zero out ci==0 (duplicate class 15)
            bd4h_f = cpool.tile([P, RB], FP32, tag="bd4h_f")
            nc.vector.tensor_copy(bd4h_f[:, :], bd4_f[:, :])
            nc.gpsimd.affine_select(bd4h_f[:, :], bd4h_f[:, :],
                                    pattern=[[-4, RB]], base=-1,
                                    channel_multiplier=1,
                                    compare_op=OP.is_ge, fill=0.0)
            bd4h = cpool.tile([P, RB], BF16, tag="bd4h")
            nc.vector.tensor_copy(bd4h[:, :], bd4h_f[:, :])
            cmod4 = _mod_col(nc, cpool, P, 4, bd4_f)      # k % 4 (f32)
            ccols = []
            for cg in range(NG):
                ccf = cpool.tile([P, 1], FP32, tag=f"ccf_cg{cg}")
                nc.vector.tensor_scalar_add(ccf[:, :], cmod4[:, :],
                                            float(CB[cg]))
                cc = cpool.tile([P, 1], U8, tag=f"ccol_cg{cg}")
                nc.vector.tensor_copy(cc[:, :], ccf[:, :])
                ccols.append(cc)

            # ---- target: load once, convert to uint8 ----
            t_i32 = tpool.tile([P, free_total], I32)
            nc.sync.dma_start(out=t_i32[:, :], in_=tgt_d[:, :])
            t_u8 = tpool.tile([P, free_total], U8)
            nc.vector.tensor_copy(t_u8[:, :], t_i32[:, :])

            # ---- main loop ----
            for s in range(S):
                psum_se = psacc.tile([P, F], FP32, tag="psum_se")
                psum_pk = psacc.tile([P, F], FP32, tag="psum_pk")
                for q in range(P // RB):
                    b0 = RB * q
                    tsl = t_u8[b0:b0 + RB, s * F:(s + 1) * F]
                    trep = epool.tile([P, F], U8, tag="trep")
                    nc.gpsimd.dma_start(
                        out=trep[:, :],
                        in_=tsl.unsqueeze(1).broadcast_to((RB, 4, F)))

                    predt = predpool.tile([P, NG * F], FP32, tag="predt")
                    for cg in range(NG):
                        nc.sync.dma_start(out=predt[:, cg * F:(cg + 1) * F],
                                          in_=vg[cg][q, :, :, s, :])

                    e_t = epool.tile([P, NG * F], BF16, tag="e")
                    nc.scalar.activation(e_t[:, :], predt[:, :], AF.Exp)

                    prod = epool.tile([P, NG * F], BF16, tag="prod")
                    for cg in range(NG):
                        nc.vector.scalar_tensor_tensor(
                            out=prod[:, cg * F:(cg + 1) * F],
                            in0=trep[:, :], scalar=ccols[cg][:, :],
                            in1=predt[:, cg * F:(cg + 1) * F],
                            op0=OP.is_equal, op1=OP.mult)

                    for cg in range(NG):
                        nc.tensor.matmul(psum_se[b0:b0 + RB, :],
                                         (bd4h if cg == NG - 1 else bd4)[:, :],
                                         e_t[:, cg * F:(cg + 1) * F],
                                         start=(cg == 0), stop=(cg == NG - 1),
                                         tile_position=(0, b0),
                                         skip_group_check=True)
                    for cg in range(NG):
                        nc.tensor.matmul(psum_pk[b0:b0 + RB, :],
                                         (bd4h if cg == NG - 1 else bd4)[:, :],
                                         prod[:, cg * F:(cg + 1) * F],
                                         start=(cg == 0), stop=(cg == NG - 1),
                                         tile_position=(0, b0),
                                         skip_group_check=True)

                lse_t = opool.tile([P, F], FP32, tag="lse")
                nc.scalar.activation(lse_t[:, :], psum_se[:, :], AF.Ln)
                loss_t = opool.tile([P, F], FP32, tag="loss")
                nc.vector.tensor_sub(loss_t[:, :], lse_t[:, :], psum_pk[:, :])
                nc.scalar.dma_start(out=loss_d[:, s * F:(s + 1) * F],
                                    in_=loss_t[:, :])
    if cap_waits:
        _cap_sync_waits(nc)
    return nc


def build_stats_nc(free_total: int = FT, cap_waits: bool = True):
    """Masked sum + count at a shared threshold: loss [P, FT] f32,
    thr [P, 1] f32 -> stats [P, 2] f32 (per-partition sum, count)."""
    nc = bass.Bass()
    loss_d = nc.dram_tensor("loss", [P, free_total], FP32, kind="ExternalInput")
    thr_d = nc.dram_tensor("thr", [P, 1], FP32, kind="ExternalInput")
    stats_d = nc.dram_tensor("stats", [P, 2], FP32, kind="ExternalOutput")

    with TileContext(nc) as tc:
        with tc.tile_pool(name="sbuf", bufs=1) as pool:
            lt = pool.tile([P, free_total], FP32)
            nc.sync.dma_start(out=lt[:, :], in_=loss_d[:, :])
            th = pool.tile([P, 1], FP32)
            nc.sync.dma_start(out=th[:, :], in_=thr_d[:, :])
            ones_t = pool.tile([P, free_total], FP32)
            nc.vector.memset(ones_t[:, :], 1.0)
            stats_t = pool.tile([P, 2], FP32)
            masked = pool.tile([P, free_total], FP32)
            nc.vector.scalar_tensor_tensor(
                out=masked[:, :], in0=lt[:, :], scalar=th[:, :], in1=lt[:, :],
                op0=OP.is_ge, op1=OP.mult, accum_out=stats_t[:, 0:1])
            mask2 = pool.tile([P, free_total], FP32)
            nc.vector.scalar_tensor_tensor(
                out=mask2[:, :], in0=lt[:, :], scalar=th[:, :], in1=ones_t[:, :],
                op0=OP.is_ge, op1=OP.mult, accum_out=stats_t[:, 1:2])
            nc.sync.dma_start(out=stats_d[:, :], in_=stats_t[:, :])
    if cap_waits:
        _cap_sync_waits(nc)
    return nc


_CACHE: dict = {}


def _programs():
    if "ce" not in _CACHE:
        _CACHE["ce"] = build_ce_nc()
        _CACHE["stats"] = build_stats_nc()
    return _CACHE["ce"], _CACHE["stats"]


def kernel(pred, target, step):
    pred = np.asarray(pred)
    target = np.asarray(target)
    tgt_i32 = target.astype(np.int32, copy=False)
    b, c, h, w = pred.shape
    assert (b, c, h, w) == (B, C, H, W)
    num = int(K_FRAC * b * h * w * max(MOMENTUM ** int(step), K_FRAC))

    nc_ce, nc_stats = _programs()
    cores = list(range(B))

    in_maps = [
        {
            "pred": np.ascontiguousarray(pred[i].reshape(C, H * W)),
            "tgt": np.ascontiguousarray(tgt_i32[i].reshape(P, FT)),
        }
        for i in cores
    ]
    r1 = run_bass_kernel_spmd(nc_ce, in_maps, cores)
    loss_shards = [r1.results[i]["loss"] for i in cores]

    loss_all = np.concatenate([ls.reshape(-1) for ls in loss_shards])
    n = loss_all.size
    tk = np.partition(loss_all, n - num)[n - num]

    thr = np.full((P, 1), tk, dtype=np.float32)
    in_maps2 = [{"loss": loss_shards[i], "thr": thr} for i in cores]
    r2 = run_bass_kernel_spmd(nc_stats, in_maps2, cores)

    tot = 0.0
    cnt = 0.0
    for i in cores:
        st = r2.results[i]["stats"].astype(np.float64)
        tot += st[:, 0].sum()
        cnt += st[:, 1].sum()
    return np.asarray(np.float32(tot / cnt))
